# revision 23
# baseline (speedup 1.0000x reference)
"""Causal self-attention (GPT-NeoX RoPE) Trainium2 kernel.

Sharding: 8 cores = 2 (batch) x 4 (head groups of 4 heads), tensor-parallel
over heads: Wqkv column-sharded, Wout row-sharded; per-core partial outputs
are reduced on the host (the TP "collective" of full_io mode).

Per-core dataflow (transpose-free by construction):
  qkvT[col, t] = Wqkv_shard.T @ x.T        (PE, fp32r, K=C chunks of 128)
  RoPE on qT/kT chunks: partition-swapped copy + mul/add (DVE/GPSIMD/ACT)
  v[t, d] from vT via PE transpose (ones column appended for row sums)
  scoresT[j, i] = kT.T @ qT                (PE, K=64, per 128-key block,
                                            causally skipped)
  expT = exp(scoresT * 1/8)                (ACT, PSUM -> SBUF)
  outT[d, i] + sums[i] = v_ext.T @ expT    (PE, accumulated over key blocks)
  outT *= 1/sums  (recip + 1-partition ones-matmul broadcast + DVE mul)
  y[t, c] = outT.T @ Wout_shard            (PE, K=256 in 2 chunks)
"""

import numpy as np

import concourse.bass as bass
import concourse.mybir as mybir
import concourse.tile as tile
from concourse.vector_clock import ScopedClock

F32 = mybir.dt.float32
F32R = mybir.dt.float32r

B, T, C = 2, 2048, 1024
H, D = 16, 64
H_LOC = H // 4  # heads per core
CH = C // 128  # contraction chunks for the qkv projection
QKV_COLS = 3 * H_LOC * D  # 768
IT_W = 512  # query-tile width
IT_N = T // IT_W  # 4
JB_N = T // 128  # 16 key blocks
ROPE_BASE = 10000.0

_MAX_WAITS = 1


def _split_sync_waits(nc, cap=_MAX_WAITS):
    """This container's walrus rejects instructions carrying more than one
    sem wait; move excess waits onto same-engine NOPs placed just before."""
    for fn in nc.m.functions:
        for bb in fn.blocks:
            out = []
            changed = False
            for inst in bb.instructions:
                si = inst.sync_info
                waits = list(si.on_wait) if (si and si.on_wait) else []
                if len(waits) > cap:
                    si.on_wait = waits[:cap]
                    rest = waits[cap:]
                    for i in range(0, len(rest), cap):
                        out.append(
                            mybir.InstNoOp(
                                name=nc.get_next_instruction_name(),
                                sync_info=mybir.SyncInfo(
                                    on_wait=rest[i : i + cap], on_update=[]
                                ),
                                bass_nofuse=True,
                                engine=inst.engine,
                            )
                        )
                    changed = True
                out.append(inst)
            if changed:
                bb.instructions[:] = out


class _TC(tile.TileContext):
    """TileContext whose exit drain never carries >1 sem wait."""

    def _drain_and_barrier(self, tick_clock, wait_clock):
        drain_inst = self.nc.sync.drain()
        wait_clock.add_sem_waits(
            drain_inst.ins, ScopedClock({None: tick_clock.global_clock})
        )
        si = drain_inst.ins.sync_info
        waits = list(si.on_wait or [])
        if len(waits) > _MAX_WAITS:
            si.on_wait = waits[:_MAX_WAITS]
            for i in range(_MAX_WAITS, len(waits), _MAX_WAITS):
                nop = self.nc.sync.nop(nofuse=True, hint="drain_wait_split")
                nop.ins.sync_info = mybir.SyncInfo(
                    on_wait=waits[i : i + _MAX_WAITS], on_update=[]
                )
        self.nc.all_engine_barrier()
        popped = self.nc._tile_sem_poison_stack.pop()
        assert popped is self._sem_poison
        self.nc.clear_and_free_semaphores(list(self.sems.allocated().values()))
        self.nc.all_engine_barrier()


def _emit_body(nc, tc, pools, io):
    """Emit one full forward pass, fully interleaved per T-quarter:
    qkv(tt) -> rope(tt) -> v-trans(tt) -> attention(it=tt) -> yproj(tt)."""
    xT, wq, wo, cosr, sinr, tri, ident, onesc, y = io
    consts = pools["consts"]
    work_exp = pools["wexp"]
    work_rot = pools["wrot"]
    work_y = pools["wy"]
    work_sm = pools["wsm"]
    qkv_ctx = pools["qkv"]
    w_ctx = pools["w"]
    x_ctx = pools["x"]
    live = pools["live"]

    # ---- load inputs: x quarter 0 and Wqkv first, chunked so the first
    # matmul can start after one chunk pair (~2us) instead of the full 7MB ----
    xT_r = xT.rearrange("(c p) t -> p c t", p=128)
    wq_r = wq.rearrange("(c p) n -> p c n", p=128)
    w_chunks = []
    xq0_chunks = []
    for ch in range(CH):
        wc = w_ctx.tile([128, QKV_COLS], F32R, tag=f"w{ch}", name=f"w{ch}")
        nc.sync.dma_start(out=wc, in_=wq_r[:, ch, :])
        xc = x_ctx.tile([128, IT_W], F32R, tag="xq", name=f"xq0_{ch}")
        nc.sync.dma_start(out=xc, in_=xT_r[:, ch, 0:IT_W])
        w_chunks.append(wc)
        xq0_chunks.append(xc)

    # ---- remaining constants ----
    wo_sb = consts.tile([128, 2, C], F32R, tag="wo")
    cos_sb = consts.tile([128, T], F32R, tag="cos")
    sin_sb = consts.tile([128, T], F32R, tag="sin")
    tri_sb = consts.tile([128, 4, IT_W], F32R, tag="tri")
    id_sb = consts.tile([128, 64], F32R, tag="id")
    ones_sb = consts.tile([128, 64], F32R, tag="ones")
    nc.sync.dma_start(out=wo_sb, in_=wo.rearrange("(c p) n -> p c n", p=128))
    nc.sync.dma_start(out=cos_sb, in_=cosr[:, :])
    nc.sync.dma_start(out=sin_sb, in_=sinr[:, :])
    nc.sync.dma_start(out=tri_sb, in_=tri.rearrange("p (r i) -> p r i", r=4))
    nc.sync.dma_start(out=id_sb, in_=ident[:, :])
    nc.sync.dma_start(out=ones_sb, in_=onesc[:, :])

    qkvT_sb = qkv_ctx.tile([128, 6, T], F32R, tag="qkvT")
    v_sb = live.tile([128, JB_N, H_LOC, 65], F32R, tag="v")
    nc.sync.dma_start(
        out=v_sb[:, :, :, 64:65],
        in_=onesc.rearrange("p (j h) -> p j h", j=JB_N).unsqueeze(3),
    )
    oT_sb = live.tile([128, 2, T], F32R, tag="oT")

    ps_sc_pool = tc.tile_pool(name="pssc", bufs=2, space="PSUM")
    ps_sc = ps_sc_pool.__enter__()
    ps_pv_pool = tc.tile_pool(name="pspv", bufs=2, space="PSUM")
    ps_pv = ps_pv_pool.__enter__()
    ps_misc_pool = tc.tile_pool(name="psmisc", bufs=2, space="PSUM")
    ps_misc = ps_misc_pool.__enter__()

    for tt in range(IT_N):
        t0 = tt * IT_W
        tsl = slice(t0, t0 + IT_W)
        if tt == 0:
            xq_chunks = xq0_chunks
        else:
            xq_chunks = []
            for ch in range(CH):
                xc = x_ctx.tile([128, IT_W], F32R, tag="xq", name=f"xq{tt}_{ch}")
                nc.sync.dma_start(out=xc, in_=xT_r[:, ch, tsl])
                xq_chunks.append(xc)

        # ---- qkv projection for this quarter ----
        for mp in range(3):
            ps = ps_sc.tile([128, 2, IT_W], F32, tag="sc", name=f"qkvps{tt}_{mp}")
            for half in range(2):
                m = 2 * mp + half
                for ch in range(CH):
                    nc.tensor.matmul(
                        ps[:, half, :],
                        lhsT=w_chunks[ch][:, m * 128 : (m + 1) * 128],
                        rhs=xq_chunks[ch][:],
                        start=(ch == 0),
                        stop=(ch == CH - 1),
                    )
            dst = qkvT_sb[:, 2 * mp : 2 * mp + 2, tsl]
            if (mp + tt) % 2 == 0:
                nc.vector.tensor_copy(dst, ps[:])
            else:
                nc.scalar.copy(dst, ps[:])

        # ---- RoPE in place on this quarter (q01, k01 first) ----
        for src_ck in (0, 2, 1, 3):
            rot = work_rot.tile([128, IT_W], F32R, tag="rot", name=f"rot{tt}{src_ck}")
            nc.vector.tensor_copy(rot[0:32, :], qkvT_sb[32:64, src_ck, tsl])
            nc.scalar.copy(rot[32:64, :], qkvT_sb[0:32, src_ck, tsl])
            nc.vector.tensor_copy(rot[64:96, :], qkvT_sb[96:128, src_ck, tsl])
            nc.scalar.copy(rot[96:128, :], qkvT_sb[64:96, src_ck, tsl])
            nc.gpsimd.tensor_mul(rot[:], rot[:], sin_sb[:, tsl])
            nc.vector.tensor_mul(
                qkvT_sb[:, src_ck, tsl], qkvT_sb[:, src_ck, tsl], cos_sb[:, tsl]
            )
            nc.vector.tensor_add(
                qkvT_sb[:, src_ck, tsl], qkvT_sb[:, src_ck, tsl], rot[:]
            )

        # ---- v transpose for this quarter's key blocks ----
        for jb in range(4 * tt, 4 * tt + 4):
            for h in range(H_LOC):
                pr = 64 * (h % 2)
                tr_ps = ps_misc.tile([128, IT_W], F32R, tag="misc", name=f"tr{jb}{h}")
                nc.tensor.transpose(
                    tr_ps[:, 0:64],
                    qkvT_sb[pr : pr + 64, 4 + h // 2, jb * 128 : (jb + 1) * 128],
                    id_sb[pr : pr + 64, 0:64],
                )
                nc.vector.tensor_copy(v_sb[:, jb, h, 0:64], tr_ps[:, 0:64])

        # ---- attention for query quarter it = tt, all heads ----
        it = tt
        i0 = it * IT_W
        isl = tsl
        jb_max = 4 * (it + 1)
        for h in range(H_LOC):
            pr = 64 * (h % 2)
            ck = h // 2
            pv_ps = ps_pv.tile([128, IT_W], F32, tag="pv", name=f"pv{it}{h}")
            for jp in range(jb_max // 2):  # paired key blocks
                sc_ps = ps_sc.tile(
                    [128, 2, IT_W], F32, tag="sc", name=f"sc{it}{h}{jp}"
                )
                for half in range(2):
                    jb = 2 * jp + half
                    # columns i < jb*128 are fully masked; trim, keeping
                    # the moving dim >= 256 (fp32r full-rate threshold)
                    trim = min(max(0, (jb - 4 * it) * 128), IT_W - 256)
                    nc.tensor.matmul(
                        sc_ps[:, half, trim:],
                        lhsT=qkvT_sb[
                            pr : pr + 64, 2 + ck, jb * 128 : (jb + 1) * 128
                        ],
                        rhs=qkvT_sb[pr : pr + 64, ck, i0 + trim : i0 + IT_W],
                        start=True,
                        stop=True,
                    )
                expT = work_exp.tile(
                    [128, 2, IT_W], F32R, tag="expT", name=f"expT{it}{h}{jp}"
                )
                nc.scalar.activation(
                    expT[:], sc_ps[:], mybir.ActivationFunctionType.Exp, scale=0.125
                )
                r0 = 2 * jp - 4 * it
                if r0 >= 0:  # diagonal pair: mask both halves in one op
                    nc.vector.tensor_mul(
                        expT[:], expT[:], tri_sb[:, r0 : r0 + 2, :]
                    )
                for half in range(2):
                    jb = 2 * jp + half
                    nc.tensor.matmul(
                        pv_ps[0:65, :],
                        lhsT=v_sb[:, jb, h, :],
                        rhs=expT[:, half, :],
                        start=(jb == 0),
                        stop=(jb == jb_max - 1),
                    )
            # normalize: outT[d, i] = pv[d, i] / pv[64, i]
            recip = work_sm.tile([1, IT_W], F32R, tag="recip", name=f"rc{it}{h}")
            with nc.allow_low_precision(reason="softmax recip rounded to f32r"):
                nc.vector.reciprocal(recip[:], pv_ps[64:65, :])
            bc_ps = ps_misc.tile([128, IT_W], F32, tag="misc", name=f"bc{it}{h}")
            nc.tensor.matmul(
                bc_ps[0:64, :],
                lhsT=ones_sb[0:1, :],
                rhs=recip[:],
                start=True,
                stop=True,
            )
            rec64 = work_sm.tile([64, IT_W], F32R, tag="rec64", name=f"r64{it}{h}")
            nc.scalar.copy(rec64[:], bc_ps[0:64, :])
            nc.vector.tensor_mul(
                oT_sb[pr : pr + 64, ck, isl], pv_ps[0:64, :], rec64[:]
            )

        # ---- output projection for this quarter's rows ----
        for tt2 in range(4 * it, 4 * it + 4):
            for cc in range(2):
                ps = ps_misc.tile(
                    [128, IT_W], F32, tag="misc", name=f"y{tt2}_{cc}"
                )
                for ck2 in range(2):
                    nc.tensor.matmul(
                        ps[:],
                        lhsT=oT_sb[:, ck2, tt2 * 128 : (tt2 + 1) * 128],
                        rhs=wo_sb[:, ck2, cc * IT_W : (cc + 1) * IT_W],
                        start=(ck2 == 0),
                        stop=(ck2 == 1),
                    )
                ysb = work_y.tile([128, IT_W], F32, tag="y", name=f"ysb{tt2}_{cc}")
                if (tt2 * 2 + cc) % 2 == 0:
                    nc.vector.tensor_copy(ysb[:], ps[:])
                else:
                    nc.scalar.copy(ysb[:], ps[:])
                nc.sync.dma_start(
                    out=y[tt2 * 128 : (tt2 + 1) * 128, cc * IT_W : (cc + 1) * IT_W],
                    in_=ysb[:],
                )

    ps_misc_pool.__exit__(None, None, None)
    ps_pv_pool.__exit__(None, None, None)
    ps_sc_pool.__exit__(None, None, None)


def build(reps=1):
    """Build the Bass program. reps>1 re-emits the body (for timing)."""
    from contextlib import ExitStack

    nc = bass.Bass("TRN2", target_bir_lowering=False, debug=False, num_devices=8)
    xT = nc.dram_tensor("xT", [C, T], F32R, kind="ExternalInput")
    wq = nc.dram_tensor("wq", [C, QKV_COLS], F32R, kind="ExternalInput")
    wo = nc.dram_tensor("wo", [H_LOC * D, C], F32R, kind="ExternalInput")
    cosr = nc.dram_tensor("cosr", [128, T], F32R, kind="ExternalInput")
    sinr = nc.dram_tensor("sinr", [128, T], F32R, kind="ExternalInput")
    tri = nc.dram_tensor("tri", [128, 4 * IT_W], F32R, kind="ExternalInput")
    ident = nc.dram_tensor("ident", [128, 64], F32R, kind="ExternalInput")
    onesc = nc.dram_tensor("onesc", [128, 64], F32R, kind="ExternalInput")
    y = nc.dram_tensor("y", [T, C], F32, kind="ExternalOutput")
    io = (xT, wq, wo, cosr, sinr, tri, ident, onesc, y)

    with _TC(nc, pool_alloc_mode="queue") as tc:
        with ExitStack() as ctx:
            pools = {
                "consts": ctx.enter_context(tc.tile_pool(name="consts", bufs=1)),
                "wexp": ctx.enter_context(tc.tile_pool(name="wexp", bufs=4)),
                "wrot": ctx.enter_context(tc.tile_pool(name="wrot", bufs=2)),
                "wy": ctx.enter_context(tc.tile_pool(name="wy", bufs=2)),
                "wsm": ctx.enter_context(tc.tile_pool(name="wsm", bufs=2)),
                "qkv": ctx.enter_context(tc.tile_pool(name="qkv", bufs=1)),
                "w": ctx.enter_context(tc.tile_pool(name="w", bufs=1)),
                "x": ctx.enter_context(tc.tile_pool(name="x", bufs=16)),
                "live": ctx.enter_context(tc.tile_pool(name="live", bufs=1)),
            }
            for _ in range(reps):
                _emit_body(nc, tc, pools, io)
    _split_sync_waits(nc)
    return nc


def make_inputs(x, Wqkv, Wout):
    """Host-side shard/layout prep. Returns in_maps for 8 cores."""
    x = np.asarray(x, dtype=np.float32)
    Wqkv = np.asarray(Wqkv, dtype=np.float32)
    Wout = np.asarray(Wout, dtype=np.float32)

    t = np.arange(T, dtype=np.float32)
    inv_freq = 1.0 / (ROPE_BASE ** (np.arange(0, D, 2, dtype=np.float32) / D))
    freqs = t[:, None] * inv_freq[None, :]  # [T, 32]
    emb = np.concatenate([freqs, freqs], axis=-1)  # [T, 64]
    cos = np.cos(emb).astype(np.float32).T  # [64, T]
    sin = np.sin(emb).astype(np.float32).T  # [64, T]
    sin_signed = np.concatenate([-sin[0:32], sin[32:64]], axis=0)
    cosr_np = np.ascontiguousarray(np.concatenate([cos, cos], axis=0))
    sinr_np = np.ascontiguousarray(np.concatenate([sin_signed, sin_signed], axis=0))

    jl = np.arange(128)
    il = np.arange(IT_W)
    tri_np = np.concatenate(
        [
            (jl[:, None] <= (il[None, :] - 128 * r)).astype(np.float32)
            for r in range(4)
        ],
        axis=1,
    )  # [128, 4*512]
    id_np = np.ascontiguousarray(np.tile(np.eye(64, dtype=np.float32), (2, 1)))
    ones_np = np.ones((128, 64), dtype=np.float32)

    in_maps = []
    for core in range(8):
        b, hg = core // 4, core % 4
        xT_np = np.ascontiguousarray(x[b].T)  # [C, T]
        cols = []
        for part in range(3):  # q, k, v
            c0 = part * (H * D) + hg * (H_LOC * D)
            cols.append(Wqkv[:, c0 : c0 + H_LOC * D])
        wq_np = np.ascontiguousarray(np.concatenate(cols, axis=1))  # [C, 768]
        wo_np = np.ascontiguousarray(
            Wout[hg * H_LOC * D : (hg + 1) * H_LOC * D, :]
        )  # [256, C]
        in_maps.append(
            {
                "xT": xT_np,
                "wq": wq_np,
                "wo": wo_np,
                "cosr": cosr_np,
                "sinr": sinr_np,
                "tri": tri_np,
                "ident": id_np,
                "onesc": ones_np,
            }
        )
    return in_maps


def run(nc, in_maps):
    from concourse.bass_utils import run_bass_kernel_spmd

    res = run_bass_kernel_spmd(nc, in_maps, core_ids=list(range(8)))
    return res


def kernel(x, Wqkv, Wout):
    nc = build()
    in_maps = make_inputs(x, Wqkv, Wout)
    res = run(nc, in_maps)
    ys = [res.results[c]["y"] for c in range(8)]
    out = np.stack(
        [ys[0] + ys[1] + ys[2] + ys[3], ys[4] + ys[5] + ys[6] + ys[7]], axis=0
    )
    return out.astype(np.float32)
